# revision 2
# baseline (speedup 1.0000x reference)
"""Bass/Tile TRN2 kernel for nn_CRMF_35296041239144 — v7 (v5 + startup/drain overlap: multi-queue input DMAs, quartered ph load, split final out DMA).

Social-LSTM-style decoder: mapping MLP on K x B hidden states, then a
12-step LSTM recurrence (hard-sigmoid gates, clipped cell) with a 2-dim
output projection per step. State transposed [H=128 partitions, rows].

v3 changes vs v2 (which was DVE 84% / ACT 83% busy in CoreSim):
- PSUM split (f,i) | (o,g), 1 bank each, triple-buffered; out-proj pso
  pair-batched (1 bank, x2). 3+3+2 = 8 banks.
- Two custom DVE ops consume o and g STRAIGHT from PSUM (no evac op):
    T2CLAMPG: t2 = min(i,1) * clamp(g_psum, -1, 1)
    HOUT:     h  = clamp(c',-1,1) * clamp01(o_psum)
  ACT only evacuates (f,i) with fused Relu -> its load halves.
- GPSIMD: c' = t1 + t2 (its only op; it was 26% busy).
- out-proj evac batched per chunk-pair (FD 512 on ACT).
"""

import numpy as np
import ml_dtypes
from contextlib import nullcontext

import concourse.bass as bass
import concourse.bacc as bacc
import concourse.tile as tile
from concourse import mybir
from concourse import dve_ops as _dve_ops
from concourse.dve_spec import Spec, Src0, Src1, C0, C1, minn, maxx, relu
from concourse.bass_utils import run_bass_kernel_spmd
import concourse.bass_utils as _bass_utils

# let walrus hoist/merge LDWEIGHTS (off by default; all stationaries are
# f32r self-loading matmuls so ldw-opt applies; measured significant)
if not getattr(_bass_utils, "_ldw_opt_patched", False):
    _orig_run_command = _bass_utils.run_command

    def _run_command_ldw(cmd, **kw):
        if isinstance(cmd, list):
            cmd = ["--enable-ldw-opt=true" if c == "--enable-ldw-opt=false"
                   else c for c in cmd]
        return _orig_run_command(cmd, **kw)

    _bass_utils.run_command = _run_command_ldw
    _bass_utils._ldw_opt_patched = True


def _register_dve_op(name, body, reference):
    """Register a custom DVE op at runtime (sha self-pinned)."""
    for o in _dve_ops.OPS:
        if o.name == name:
            return o
    from concourse.dve_spec import lower
    from concourse.dve_uop import DveOpSpec
    spec = Spec(body=body, reference=reference)
    row = max(_dve_ops._SUB_OPCODE_FOR_NAME.values()) + 1
    assert row < 0x20
    _dve_ops._SUB_OPCODE_FOR_NAME[name] = row
    shas = {}
    for v in ("v3", "v4"):
        shas[v] = DveOpSpec(name=name, opcode=row, uops=lower(spec, ver=v),
                            rd1_en=True).sha(v)
    op = _dve_ops.DveOp(name, spec, subdim=False, uops_sha=shas)
    _dve_ops.OPS.append(op)
    _dve_ops.CUSTOM_DVE_SPECS[name] = spec
    return op


# t2 = min(i, 1) * clamp(g_psum, -1, 1)   [s0=-1, s1=1]
T2OP = _register_dve_op(
    "T2CLAMPG_ANT",
    minn(Src0, C1) * minn(maxx(Src1, C0), C1),
    lambda in0, in1, s0, s1, imm2: np.minimum(in0.astype(np.float32), s1)
    * np.minimum(np.maximum(in1.astype(np.float32), s0), s1),
)
# h = clamp(c', -1, 1) * clamp01(o_psum)  [s0=-1, s1=1]
HOP = _register_dve_op(
    "HOUT_ANT",
    minn(maxx(Src0, C0), C1) * minn(relu(Src1), C1),
    lambda in0, in1, s0, s1, imm2: np.minimum(
        np.maximum(in0.astype(np.float32), s0), s1)
    * np.minimum(np.maximum(in1.astype(np.float32), 0.0), s1),
)

OBS_LEN, K, B, H, MID, NC_OUT, CIN = 12, 20, 2048, 128, 256, 2, 3
NCORES = 8
BC = B // NCORES            # 256 batch rows per core
ROWS = K * BC               # 5120 rows per core (k-major: r = k*BC + b)
CHUNK = 256
NCH = ROWS // CHUNK         # 20
NTILE = ROWS // 128         # 40 transpose tiles

F32 = mybir.dt.float32
F32R = mybir.dt.float32r
BF16 = mybir.dt.bfloat16
AF = mybir.ActivationFunctionType
OP = mybir.AluOpType

# device gate order: (f, i) in P_fi, (o, g) in P_og.
# reference block order in w_ih/w_hh is [i, f, g, o].
GATE_SRC = [1, 0, 3, 2]     # device gate idx -> source block


def build_nc(reps: int = 1):
    nc = bacc.Bacc("TRN2", target_bir_lowering=False, debug=False)

    ph = nc.dram_tensor("ph", [ROWS, H], F32R, kind="ExternalInput")
    whh = nc.dram_tensor("whh", [H, 4 * H], F32R, kind="ExternalInput")
    wblk = nc.dram_tensor("wblk", [128, H], F32R, kind="ExternalInput")
    xmf = nc.dram_tensor("xmf", [128, OBS_LEN, 2 * CHUNK], F32R,
                         kind="ExternalInput")
    xmg = nc.dram_tensor("xmg", [128, OBS_LEN, 2 * CHUNK], F32R,
                         kind="ExternalInput")
    w0 = nc.dram_tensor("w0", [H, MID], F32R, kind="ExternalInput")
    w1 = nc.dram_tensor("w1", [MID, H], F32R, kind="ExternalInput")
    oww = nc.dram_tensor("oww", [H, NC_OUT], F32R, kind="ExternalInput")
    bpack = nc.dram_tensor("bpack", [128, 4], F32, kind="ExternalInput")
    ident = nc.dram_tensor("ident", [128, 128], F32R, kind="ExternalInput")
    outd = nc.dram_tensor("out", [OBS_LEN, NC_OUT, ROWS], F32,
                          kind="ExternalOutput")

    with tile.TileContext(nc) as tc:
        with tc.tile_pool(name="const", bufs=1) as const, \
             tc.tile_pool(name="state", bufs=1) as state, \
             tc.tile_pool(name="outs", bufs=2) as outs_p:

            whh_sb = const.tile([128, 4 * H], F32R)
            nc.sync.dma_start(out=whh_sb[:], in_=whh[:])
            w0_sb = const.tile([128, MID], F32R)
            nc.sync.dma_start(out=w0_sb[:], in_=w0[:])
            w1_sb = const.tile([128, 2, H], F32R)
            nc.sync.dma_start(out=w1_sb[:],
                              in_=w1.rearrange("(a p) h -> p a h", p=128))
            oww_sb = const.tile([128, NC_OUT], F32R)
            nc.sync.dma_start(out=oww_sb[:], in_=oww[:])
            bp_sb = const.tile([128, 4], F32)
            nc.sync.dma_start(out=bp_sb[:], in_=bpack[:])
            id_sb = const.tile([128, 128], F32R)
            nc.sync.dma_start(out=id_sb[:], in_=ident[:])
            wblk_sb = const.tile([128, H], F32R)
            nc.sync.dma_start(out=wblk_sb[:], in_=wblk[:])
            xmf_sb = const.tile([128, OBS_LEN, 2 * CHUNK], F32R)
            nc.scalar.dma_start(out=xmf_sb[:], in_=xmf[:])
            xmg_sb = const.tile([128, OBS_LEN, 2 * CHUNK], F32R)
            nc.gpsimd.dma_start(out=xmg_sb[:], in_=xmg[:])

            # per-chunk state tiles so chunk pipelines stay independent
            h_ch = [state.tile([128, CHUNK], F32R, name=f"h{j}",
                               tag=f"h{j}") for j in range(NCH)]
            c_ch = [state.tile([128, CHUNK], BF16, name=f"c{j}",
                               tag=f"c{j}") for j in range(NCH)]

            with (tc.For_i(0, reps, 1) if reps > 1 else nullcontext()):
                # ---------- phase 1: transpose ph, mapping MLP ----------
                with tc.tile_pool(name="mlpsb", bufs=1) as mlpsb, \
                     tc.tile_pool(name="h1p", bufs=3) as h1p, \
                     tc.tile_pool(name="pst", bufs=2, space="PSUM") as pst, \
                     tc.tile_pool(name="ps1", bufs=2, space="PSUM") as ps1p, \
                     tc.tile_pool(name="ps0", bufs=2, space="PSUM") as ps0p:

                    for j in range(NCH):
                        nc.vector.memset(c_ch[j][:], 0.0)

                    phr = ph.rearrange("(n p) h -> p n h", p=128)
                    nq = NTILE // 4
                    ph_nat = [mlpsb.tile([128, nq, H], F32R, name=f"ph_nat{q}",
                                         tag=f"ph_nat{q}") for q in range(4)]
                    for q in range(4):
                        nc.sync.dma_start(out=ph_nat[q][:],
                                          in_=phr[:, q * nq:(q + 1) * nq, :])
                    ph_t = mlpsb.tile([128, ROWS], F32R, tag="ph_t")
                    for n in range(NTILE):
                        ptile = pst.tile([128, 128], F32R)
                        nc.tensor.transpose(ptile[:],
                                            ph_nat[n // nq][:, n % nq, :],
                                            id_sb[:])
                        sl = ph_t[:, n * 128:(n + 1) * 128]
                        nc.vector.tensor_copy(sl, ptile[:])

                    for j in range(0, NCH, 2):
                        rs = slice(j * CHUNK, (j + 2) * CHUNK)
                        ps1 = ps1p.tile([128, 2, 2 * CHUNK], F32)
                        nc.tensor.matmul(ps1[:, 0, :], w0_sb[:, 0:128],
                                         ph_t[:, rs], start=True, stop=True)
                        nc.tensor.matmul(ps1[:, 1, :], w0_sb[:, 128:256],
                                         ph_t[:, rs], start=True, stop=True)
                        h1t = h1p.tile([128, 2, 2 * CHUNK], F32R, tag="h1")
                        nc.scalar.activation(h1t[:, 0, :], ps1[:, 0, :],
                                             AF.Lrelu, bias=bp_sb[:, 0:1],
                                             alpha=0.01)
                        nc.scalar.activation(h1t[:, 1, :], ps1[:, 1, :],
                                             AF.Lrelu, bias=bp_sb[:, 1:2],
                                             alpha=0.01)
                        ps0 = ps0p.tile([128, 2 * CHUNK], F32)
                        nc.tensor.matmul(ps0[:], w1_sb[:, 0, :],
                                         h1t[:, 0, :], start=True, stop=False)
                        nc.tensor.matmul(ps0[:], w1_sb[:, 1, :],
                                         h1t[:, 1, :], start=False, stop=True)
                        nc.vector.tensor_scalar(
                            out=h_ch[j][:], in0=ps0[:, 0:CHUNK],
                            scalar1=bp_sb[:, 2:3], scalar2=None, op0=OP.add)
                        nc.vector.tensor_scalar(
                            out=h_ch[j + 1][:], in0=ps0[:, CHUNK:2 * CHUNK],
                            scalar1=bp_sb[:, 2:3], scalar2=None, op0=OP.add)

                # ---------- phase 2: LSTM recurrence ----------
                with tc.tile_pool(name="pfi", bufs=3, space="PSUM") as pfi_p, \
                     tc.tile_pool(name="pog", bufs=3, space="PSUM") as pog_p, \
                     tc.tile_pool(name="pso", bufs=2, space="PSUM") as pso_p, \
                     tc.tile_pool(name="fi", bufs=3) as fi_p, \
                     tc.tile_pool(name="t1p", bufs=3) as t1_p, \
                     tc.tile_pool(name="t2p", bufs=3) as t2_p:

                    outstep = None
                    prev_outstep = None
                    pso = None
                    for t in range(OBS_LEN + 1):
                        prev_outstep = outstep
                        if t < OBS_LEN:
                            outstep = outs_p.tile([NC_OUT, ROWS], F32,
                                                  tag="outstep",
                                                  name="outstep")
                        for j in range(NCH):
                            # output projection of the PREVIOUS step; read h
                            # before this step's HOUT overwrites it
                            if t > 0:
                                if j % 2 == 0:
                                    pso = pso_p.tile([NC_OUT, 2, CHUNK], F32,
                                                     name="pso")
                                nc.tensor.matmul(pso[:, j % 2, :], oww_sb[:],
                                                 h_ch[j][:], start=True,
                                                 stop=True)
                                if j % 2 == 1:
                                    nc.scalar.activation(
                                        prev_outstep[:, (j - 1) * CHUNK:
                                                     (j + 1) * CHUNK],
                                        pso[:], AF.Identity,
                                        bias=bp_sb[0:NC_OUT, 3:4])
                            if t == OBS_LEN:
                                continue

                            pfi = pfi_p.tile([128, 2, CHUNK], F32, name="pfi")
                            pog = pog_p.tile([128, 2, CHUNK], F32, name="pog")
                            # x/bias init (block-diagonal, full-array MMs),
                            # recurrent gate MMs accumulate on top
                            nc.tensor.matmul(pfi[:], wblk_sb[:],
                                             xmf_sb[:, t, :], start=True,
                                             stop=False)
                            nc.tensor.matmul(pfi[:, 0, :], whh_sb[:, 0:128],
                                             h_ch[j][:], start=False,
                                             stop=True)
                            nc.tensor.matmul(pfi[:, 1, :], whh_sb[:, 128:256],
                                             h_ch[j][:], start=False,
                                             stop=True)
                            nc.tensor.matmul(pog[:], wblk_sb[:],
                                             xmg_sb[:, t, :], start=True,
                                             stop=False)
                            nc.tensor.matmul(pog[:, 0, :], whh_sb[:, 256:384],
                                             h_ch[j][:], start=False,
                                             stop=True)
                            nc.tensor.matmul(pog[:, 1, :], whh_sb[:, 384:512],
                                             h_ch[j][:], start=False,
                                             stop=True)

                            # evac f,i with fused relu (ACT)
                            fi = fi_p.tile([128, 2, CHUNK], BF16, tag="fi",
                                           name="fi")
                            nc.scalar.activation(fi[:], pfi[:], AF.Relu)
                            # t1 = min(f,1) * c
                            t1 = t1_p.tile([128, CHUNK], BF16, tag="t1",
                                           name="t1")
                            nc.vector.scalar_tensor_tensor(
                                out=t1[:], in0=fi[:, 0, :], scalar=1.0,
                                in1=c_ch[j][:], op0=OP.min, op1=OP.mult)
                            # t2 = min(i,1) * clamp(g) straight from PSUM
                            t2 = t2_p.tile([128, CHUNK], BF16, tag="t2",
                                           name="t2")
                            nc.vector._custom_dve(
                                T2OP, out=t2[:], in0=fi[:, 1, :],
                                in1=pog[:, 1, :], s0=-1.0, s1=1.0)
                            # c' = t1 + t2  (GPSIMD)
                            nc.gpsimd.tensor_tensor(
                                out=c_ch[j][:], in0=t1[:], in1=t2[:],
                                op=OP.add)
                            # h = clamp(c') * clamp01(o) straight from PSUM
                            nc.vector._custom_dve(
                                HOP, out=h_ch[j][:], in0=c_ch[j][:],
                                in1=pog[:, 0, :], s0=-1.0, s1=1.0)
                        if t > 0:
                            if t < OBS_LEN:
                                nc.sync.dma_start(out=outd[t - 1],
                                                  in_=prev_outstep[:])
                            else:
                                q4 = ROWS // 4
                                for qi, eng in enumerate((nc.sync,
                                                          nc.scalar,
                                                          nc.gpsimd,
                                                          nc.sync)):
                                    eng.dma_start(
                                        out=outd[t - 1, :,
                                                 qi * q4:(qi + 1) * q4],
                                        in_=prev_outstep[:, qi * q4:
                                                         (qi + 1) * q4])

    nc.finalize()
    return nc


def prep_inputs(obs_traj_rel, pred_lstm_hidden, map_w0, map_b0, map_w1,
                map_b1, w_ih, w_hh, b_ih, b_hh, out_w, out_b):
    """Host-side prep -> list of per-core input dicts."""
    f32 = np.float32
    bias = (np.asarray(b_ih, f32) + np.asarray(b_hh, f32))
    w_hh = np.asarray(w_hh, f32)
    w_ih = np.asarray(w_ih, f32)

    # device gate order (f, i, o, g); hard-sigmoid scale 1/6 + offset 0.5
    # folded into f,i,o; g unscaled. wblk rows 3g+c carry the x-term
    # (c=0,1) and bias (c=2, via the ones-row of the moving tile).
    whh_stat = np.empty((H, 4 * H), f32)
    wblk_stat = np.zeros((128, H), f32)
    for gi in range(4):
        sb = GATE_SRC[gi]
        s = (1.0 / 6.0) if gi != 3 else 1.0
        off = 0.5 if gi != 3 else 0.0
        whh_stat[:, gi * 128:(gi + 1) * 128] = \
            w_hh[sb * 128:(sb + 1) * 128].T * s
        wblk_stat[3 * gi + 0:3 * gi + 2, :] = \
            w_ih[sb * 128:(sb + 1) * 128, :].T * s
        wblk_stat[3 * gi + 2, :] = bias[sb * 128:(sb + 1) * 128] * s + off

    bpack = np.zeros((128, 4), f32)
    bpack[:, 0] = np.asarray(map_b0, f32)[0:128]
    bpack[:, 1] = np.asarray(map_b0, f32)[128:256]
    bpack[:, 2] = np.asarray(map_b1, f32)
    bpack[0:NC_OUT, 3] = np.asarray(out_b, f32)

    obs = np.asarray(obs_traj_rel, f32)
    xs = np.concatenate([obs[0:1], obs[:-1]], axis=0)[:, :, 0:2]  # [T,B,2]
    ph_full = np.asarray(pred_lstm_hidden, f32)

    common = dict(
        whh=whh_stat, wblk=wblk_stat,
        w0=np.ascontiguousarray(np.asarray(map_w0, f32)),
        w1=np.ascontiguousarray(np.asarray(map_w1, f32)),
        oww=np.ascontiguousarray(np.asarray(out_w, f32)),
        bpack=bpack, ident=np.eye(128, dtype=f32),
    )
    in_maps = []
    for c in range(NCORES):
        bs = slice(c * BC, (c + 1) * BC)
        ph_core = np.ascontiguousarray(
            ph_full[:, bs, :].reshape(ROWS, H))
        x_core = xs[:, bs, :]                       # [T, BC=256, 2]
        xt = np.empty((3, OBS_LEN, CHUNK), f32)     # x~ = (x0, x1, 1)
        xt[0] = x_core[:, :, 0]
        xt[1] = x_core[:, :, 1]
        xt[2] = 1.0
        # block-diagonal movings: (f,i) rows 0-5, (o,g) rows 6-11
        xmf_core = np.zeros((128, OBS_LEN, 2, CHUNK), f32)
        xmg_core = np.zeros((128, OBS_LEN, 2, CHUNK), f32)
        for half in range(2):
            xmf_core[3 * half:3 * half + 3, :, half, :] = xt
            xmg_core[6 + 3 * half:9 + 3 * half, :, half, :] = xt
        in_maps.append(dict(
            ph=ph_core,
            xmf=xmf_core.reshape(128, OBS_LEN, 2 * CHUNK),
            xmg=xmg_core.reshape(128, OBS_LEN, 2 * CHUNK), **common))
    return in_maps


def assemble_output(results):
    """Per-core [T, 2, ROWS] (k-major rows) -> full [T, K, B, 2]."""
    out = np.empty((OBS_LEN, K, B, NC_OUT), np.float32)
    for c, res in enumerate(results):
        o = res["out"].reshape(OBS_LEN, NC_OUT, K, BC)
        out[:, :, c * BC:(c + 1) * BC, :] = o.transpose(0, 2, 3, 1)
    return out


def kernel(**inputs):
    nc = build_nc(reps=1)
    in_maps = prep_inputs(**inputs)
    res = run_bass_kernel_spmd(nc, in_maps, core_ids=list(range(NCORES)))
    return assemble_output(res.results)


if __name__ == "__main__":
    import reference as R
    inputs = {k: np.asarray(v) for k, v in R.setup_inputs().items()}
    got = kernel(**inputs)
    import jax.numpy as jnp
    ref = np.asarray(
        R.reference(**{k: jnp.asarray(v) for k, v in inputs.items()}))
    err = np.abs(got - ref).max()
    rel = err / np.abs(ref).max()
    print(f"absmax={err:.4e} rel={rel:.4e}")


# revision 3
# speedup vs baseline: 1.9350x; 1.9350x over previous
"""Bass/Tile TRN2 kernel for nn_CRMF_35296041239144 — v7 (v5 + startup/drain overlap: multi-queue input DMAs, quartered ph load, split final out DMA).

Social-LSTM-style decoder: mapping MLP on K x B hidden states, then a
12-step LSTM recurrence (hard-sigmoid gates, clipped cell) with a 2-dim
output projection per step. State transposed [H=128 partitions, rows].

v3 changes vs v2 (which was DVE 84% / ACT 83% busy in CoreSim):
- PSUM split (f,i) | (o,g), 1 bank each, triple-buffered; out-proj pso
  pair-batched (1 bank, x2). 3+3+2 = 8 banks.
- Two custom DVE ops consume o and g STRAIGHT from PSUM (no evac op):
    T2CLAMPG: t2 = min(i,1) * clamp(g_psum, -1, 1)
    HOUT:     h  = clamp(c',-1,1) * clamp01(o_psum)
  ACT only evacuates (f,i) with fused Relu -> its load halves.
- GPSIMD: c' = t1 + t2 (its only op; it was 26% busy).
- out-proj evac batched per chunk-pair (FD 512 on ACT).
"""

import numpy as np
from contextlib import nullcontext

import concourse.bass as bass
import concourse.bacc as bacc
import concourse.tile as tile
from concourse import mybir
from concourse import dve_ops as _dve_ops
from concourse.dve_spec import Spec, Src0, Src1, C0, C1, minn, maxx, relu
from concourse.bass_utils import run_bass_kernel_spmd
import concourse.bass_utils as _bass_utils

# let walrus hoist/merge LDWEIGHTS (off by default; all stationaries are
# f32r self-loading matmuls so ldw-opt applies; measured significant)
if not getattr(_bass_utils, "_ldw_opt_patched", False):
    _orig_run_command = _bass_utils.run_command

    def _run_command_ldw(cmd, **kw):
        if isinstance(cmd, list):
            cmd = ["--enable-ldw-opt=true" if c == "--enable-ldw-opt=false"
                   else c for c in cmd]
        return _orig_run_command(cmd, **kw)

    _bass_utils.run_command = _run_command_ldw
    _bass_utils._ldw_opt_patched = True


def _register_dve_op(name, body, reference):
    """Register a custom DVE op at runtime (sha self-pinned)."""
    for o in _dve_ops.OPS:
        if o.name == name:
            return o
    from concourse.dve_spec import lower
    from concourse.dve_uop import DveOpSpec
    spec = Spec(body=body, reference=reference)
    row = max(_dve_ops._SUB_OPCODE_FOR_NAME.values()) + 1
    assert row < 0x20
    _dve_ops._SUB_OPCODE_FOR_NAME[name] = row
    shas = {}
    for v in ("v3", "v4"):
        shas[v] = DveOpSpec(name=name, opcode=row, uops=lower(spec, ver=v),
                            rd1_en=True).sha(v)
    op = _dve_ops.DveOp(name, spec, subdim=False, uops_sha=shas)
    _dve_ops.OPS.append(op)
    _dve_ops.CUSTOM_DVE_SPECS[name] = spec
    return op


# t2 = min(i, 1) * clamp(g_psum, -1, 1)   [s0=-1, s1=1]
T2OP = _register_dve_op(
    "T2CLAMPG_ANT",
    minn(Src0, C1) * minn(maxx(Src1, C0), C1),
    lambda in0, in1, s0, s1, imm2: np.minimum(in0.astype(np.float32), s1)
    * np.minimum(np.maximum(in1.astype(np.float32), s0), s1),
)
# h = clamp(c', -1, 1) * clamp01(o_psum)  [s0=-1, s1=1]
HOP = _register_dve_op(
    "HOUT_ANT",
    minn(maxx(Src0, C0), C1) * minn(relu(Src1), C1),
    lambda in0, in1, s0, s1, imm2: np.minimum(
        np.maximum(in0.astype(np.float32), s0), s1)
    * np.minimum(np.maximum(in1.astype(np.float32), 0.0), s1),
)

OBS_LEN, K, B, H, MID, NC_OUT, CIN = 12, 20, 2048, 128, 256, 2, 3
NCORES = 8
BC = B // NCORES            # 256 batch rows per core
ROWS = K * BC               # 5120 rows per core (k-major: r = k*BC + b)
CHUNK = 256
NCH = ROWS // CHUNK         # 20
NTILE = ROWS // 128         # 40 transpose tiles

F32 = mybir.dt.float32
F32R = mybir.dt.float32r
BF16 = mybir.dt.bfloat16
AF = mybir.ActivationFunctionType
OP = mybir.AluOpType

# device gate order: (f, i) in P_fi, (o, g) in P_og.
# reference block order in w_ih/w_hh is [i, f, g, o].
GATE_SRC = [1, 0, 3, 2]     # device gate idx -> source block


def build_nc(reps: int = 1):
    nc = bacc.Bacc("TRN2", target_bir_lowering=False, debug=False)

    ph = nc.dram_tensor("ph", [ROWS, H], F32R, kind="ExternalInput")
    whh = nc.dram_tensor("whh", [H, 4 * H], F32R, kind="ExternalInput")
    wblk = nc.dram_tensor("wblk", [128, H], F32R, kind="ExternalInput")
    xmf = nc.dram_tensor("xmf", [128, OBS_LEN, 2 * CHUNK], F32R,
                         kind="ExternalInput")
    xmg = nc.dram_tensor("xmg", [128, OBS_LEN, 2 * CHUNK], F32R,
                         kind="ExternalInput")
    w0 = nc.dram_tensor("w0", [H, MID], F32R, kind="ExternalInput")
    w1 = nc.dram_tensor("w1", [MID, H], F32R, kind="ExternalInput")
    oww = nc.dram_tensor("oww", [H, NC_OUT], F32R, kind="ExternalInput")
    bpack = nc.dram_tensor("bpack", [128, 4], F32, kind="ExternalInput")
    ident = nc.dram_tensor("ident", [128, 128], F32R, kind="ExternalInput")
    outd = nc.dram_tensor("out", [OBS_LEN, NC_OUT, ROWS], F32,
                          kind="ExternalOutput")

    with tile.TileContext(nc) as tc:
        with tc.tile_pool(name="const", bufs=1) as const, \
             tc.tile_pool(name="state", bufs=1) as state, \
             tc.tile_pool(name="outs", bufs=2) as outs_p:

            whh_sb = const.tile([128, 4 * H], F32R)
            nc.sync.dma_start(out=whh_sb[:], in_=whh[:])
            w0_sb = const.tile([128, MID], F32R)
            nc.sync.dma_start(out=w0_sb[:], in_=w0[:])
            w1_sb = const.tile([128, 2, H], F32R)
            nc.sync.dma_start(out=w1_sb[:],
                              in_=w1.rearrange("(a p) h -> p a h", p=128))
            oww_sb = const.tile([128, NC_OUT], F32R)
            nc.sync.dma_start(out=oww_sb[:], in_=oww[:])
            bp_sb = const.tile([128, 4], F32)
            nc.sync.dma_start(out=bp_sb[:], in_=bpack[:])
            id_sb = const.tile([128, 128], F32R)
            nc.sync.dma_start(out=id_sb[:], in_=ident[:])
            wblk_sb = const.tile([128, H], F32R)
            nc.sync.dma_start(out=wblk_sb[:], in_=wblk[:])
            xmf_sb = const.tile([128, OBS_LEN, 2 * CHUNK], F32R)
            nc.scalar.dma_start(out=xmf_sb[:], in_=xmf[:])
            xmg_sb = const.tile([128, OBS_LEN, 2 * CHUNK], F32R)
            nc.gpsimd.dma_start(out=xmg_sb[:], in_=xmg[:])

            # per-chunk state tiles so chunk pipelines stay independent
            h_ch = [state.tile([128, CHUNK], F32R, name=f"h{j}",
                               tag=f"h{j}") for j in range(NCH)]
            c_ch = [state.tile([128, CHUNK], BF16, name=f"c{j}",
                               tag=f"c{j}") for j in range(NCH)]

            with (tc.For_i(0, reps, 1) if reps > 1 else nullcontext()):
                # ---------- phase 1: transpose ph, mapping MLP ----------
                with tc.tile_pool(name="mlpsb", bufs=1) as mlpsb, \
                     tc.tile_pool(name="h1p", bufs=3) as h1p, \
                     tc.tile_pool(name="pst", bufs=2, space="PSUM") as pst, \
                     tc.tile_pool(name="ps1", bufs=2, space="PSUM") as ps1p, \
                     tc.tile_pool(name="ps0", bufs=2, space="PSUM") as ps0p:

                    for j in range(NCH):
                        nc.vector.memset(c_ch[j][:], 0.0)

                    phr = ph.rearrange("(n p) h -> p n h", p=128)
                    nq = NTILE // 4
                    ph_nat = [mlpsb.tile([128, nq, H], F32R, name=f"ph_nat{q}",
                                         tag=f"ph_nat{q}") for q in range(4)]
                    for q in range(4):
                        nc.sync.dma_start(out=ph_nat[q][:],
                                          in_=phr[:, q * nq:(q + 1) * nq, :])
                    ph_t = mlpsb.tile([128, ROWS], F32R, tag="ph_t")
                    for n in range(NTILE):
                        ptile = pst.tile([128, 128], F32R)
                        nc.tensor.transpose(ptile[:],
                                            ph_nat[n // nq][:, n % nq, :],
                                            id_sb[:])
                        sl = ph_t[:, n * 128:(n + 1) * 128]
                        nc.vector.tensor_copy(sl, ptile[:])

                    for j in range(0, NCH, 2):
                        rs = slice(j * CHUNK, (j + 2) * CHUNK)
                        ps1 = ps1p.tile([128, 2, 2 * CHUNK], F32)
                        nc.tensor.matmul(ps1[:, 0, :], w0_sb[:, 0:128],
                                         ph_t[:, rs], start=True, stop=True)
                        nc.tensor.matmul(ps1[:, 1, :], w0_sb[:, 128:256],
                                         ph_t[:, rs], start=True, stop=True)
                        h1t = h1p.tile([128, 2, 2 * CHUNK], F32R, tag="h1")
                        nc.scalar.activation(h1t[:, 0, :], ps1[:, 0, :],
                                             AF.Lrelu, bias=bp_sb[:, 0:1],
                                             alpha=0.01)
                        nc.scalar.activation(h1t[:, 1, :], ps1[:, 1, :],
                                             AF.Lrelu, bias=bp_sb[:, 1:2],
                                             alpha=0.01)
                        ps0 = ps0p.tile([128, 2 * CHUNK], F32)
                        nc.tensor.matmul(ps0[:], w1_sb[:, 0, :],
                                         h1t[:, 0, :], start=True, stop=False)
                        nc.tensor.matmul(ps0[:], w1_sb[:, 1, :],
                                         h1t[:, 1, :], start=False, stop=True)
                        nc.vector.tensor_scalar(
                            out=h_ch[j][:], in0=ps0[:, 0:CHUNK],
                            scalar1=bp_sb[:, 2:3], scalar2=None, op0=OP.add)
                        nc.vector.tensor_scalar(
                            out=h_ch[j + 1][:], in0=ps0[:, CHUNK:2 * CHUNK],
                            scalar1=bp_sb[:, 2:3], scalar2=None, op0=OP.add)

                # ---------- phase 2: LSTM recurrence ----------
                with tc.tile_pool(name="pfi", bufs=3, space="PSUM") as pfi_p, \
                     tc.tile_pool(name="pog", bufs=3, space="PSUM") as pog_p, \
                     tc.tile_pool(name="pso", bufs=2, space="PSUM") as pso_p, \
                     tc.tile_pool(name="fi", bufs=3) as fi_p, \
                     tc.tile_pool(name="t1p", bufs=3) as t1_p, \
                     tc.tile_pool(name="t2p", bufs=3) as t2_p:

                    outstep = None
                    prev_outstep = None
                    pso = None
                    for t in range(OBS_LEN + 1):
                        prev_outstep = outstep
                        if t < OBS_LEN:
                            outstep = outs_p.tile([NC_OUT, ROWS], F32,
                                                  tag="outstep",
                                                  name="outstep")
                        for j in range(NCH):
                            # output projection of the PREVIOUS step; read h
                            # before this step's HOUT overwrites it
                            if t > 0:
                                if j % 2 == 0:
                                    pso = pso_p.tile([NC_OUT, 2, CHUNK], F32,
                                                     name="pso")
                                nc.tensor.matmul(pso[:, j % 2, :], oww_sb[:],
                                                 h_ch[j][:], start=True,
                                                 stop=True)
                                if j % 2 == 1:
                                    nc.scalar.activation(
                                        prev_outstep[:, (j - 1) * CHUNK:
                                                     (j + 1) * CHUNK],
                                        pso[:], AF.Identity,
                                        bias=bp_sb[0:NC_OUT, 3:4])
                            if t == OBS_LEN:
                                continue

                            pfi = pfi_p.tile([128, 2, CHUNK], F32, name="pfi")
                            pog = pog_p.tile([128, 2, CHUNK], F32, name="pog")
                            # x/bias init (block-diagonal, full-array MMs),
                            # recurrent gate MMs accumulate on top
                            nc.tensor.matmul(pfi[:], wblk_sb[:],
                                             xmf_sb[:, t, :], start=True,
                                             stop=False)
                            nc.tensor.matmul(pfi[:, 0, :], whh_sb[:, 0:128],
                                             h_ch[j][:], start=False,
                                             stop=True)
                            nc.tensor.matmul(pfi[:, 1, :], whh_sb[:, 128:256],
                                             h_ch[j][:], start=False,
                                             stop=True)
                            nc.tensor.matmul(pog[:], wblk_sb[:],
                                             xmg_sb[:, t, :], start=True,
                                             stop=False)
                            nc.tensor.matmul(pog[:, 0, :], whh_sb[:, 256:384],
                                             h_ch[j][:], start=False,
                                             stop=True)
                            nc.tensor.matmul(pog[:, 1, :], whh_sb[:, 384:512],
                                             h_ch[j][:], start=False,
                                             stop=True)

                            # evac f,i with fused relu (ACT)
                            fi = fi_p.tile([128, 2, CHUNK], BF16, tag="fi",
                                           name="fi")
                            nc.scalar.activation(fi[:], pfi[:], AF.Relu)
                            # t1 = min(f,1) * c
                            t1 = t1_p.tile([128, CHUNK], BF16, tag="t1",
                                           name="t1")
                            nc.vector.scalar_tensor_tensor(
                                out=t1[:], in0=fi[:, 0, :], scalar=1.0,
                                in1=c_ch[j][:], op0=OP.min, op1=OP.mult)
                            # t2 = min(i,1) * clamp(g) straight from PSUM
                            t2 = t2_p.tile([128, CHUNK], BF16, tag="t2",
                                           name="t2")
                            nc.vector._custom_dve(
                                T2OP, out=t2[:], in0=fi[:, 1, :],
                                in1=pog[:, 1, :], s0=-1.0, s1=1.0)
                            # c' = t1 + t2  (GPSIMD)
                            nc.gpsimd.tensor_tensor(
                                out=c_ch[j][:], in0=t1[:], in1=t2[:],
                                op=OP.add)
                            # h = clamp(c') * clamp01(o) straight from PSUM
                            nc.vector._custom_dve(
                                HOP, out=h_ch[j][:], in0=c_ch[j][:],
                                in1=pog[:, 0, :], s0=-1.0, s1=1.0)
                        if t > 0:
                            if t < OBS_LEN:
                                nc.sync.dma_start(out=outd[t - 1],
                                                  in_=prev_outstep[:])
                            else:
                                q4 = ROWS // 4
                                for qi, eng in enumerate((nc.sync,
                                                          nc.scalar,
                                                          nc.gpsimd,
                                                          nc.sync)):
                                    eng.dma_start(
                                        out=outd[t - 1, :,
                                                 qi * q4:(qi + 1) * q4],
                                        in_=prev_outstep[:, qi * q4:
                                                         (qi + 1) * q4])

    nc.finalize()
    return nc


def prep_inputs(obs_traj_rel, pred_lstm_hidden, map_w0, map_b0, map_w1,
                map_b1, w_ih, w_hh, b_ih, b_hh, out_w, out_b):
    """Host-side prep -> list of per-core input dicts."""
    f32 = np.float32
    bias = (np.asarray(b_ih, f32) + np.asarray(b_hh, f32))
    w_hh = np.asarray(w_hh, f32)
    w_ih = np.asarray(w_ih, f32)

    # device gate order (f, i, o, g); hard-sigmoid scale 1/6 + offset 0.5
    # folded into f,i,o; g unscaled. wblk rows 3g+c carry the x-term
    # (c=0,1) and bias (c=2, via the ones-row of the moving tile).
    whh_stat = np.empty((H, 4 * H), f32)
    wblk_stat = np.zeros((128, H), f32)
    for gi in range(4):
        sb = GATE_SRC[gi]
        s = (1.0 / 6.0) if gi != 3 else 1.0
        off = 0.5 if gi != 3 else 0.0
        whh_stat[:, gi * 128:(gi + 1) * 128] = \
            w_hh[sb * 128:(sb + 1) * 128].T * s
        wblk_stat[3 * gi + 0:3 * gi + 2, :] = \
            w_ih[sb * 128:(sb + 1) * 128, :].T * s
        wblk_stat[3 * gi + 2, :] = bias[sb * 128:(sb + 1) * 128] * s + off

    bpack = np.zeros((128, 4), f32)
    bpack[:, 0] = np.asarray(map_b0, f32)[0:128]
    bpack[:, 1] = np.asarray(map_b0, f32)[128:256]
    bpack[:, 2] = np.asarray(map_b1, f32)
    bpack[0:NC_OUT, 3] = np.asarray(out_b, f32)

    obs = np.asarray(obs_traj_rel, f32)
    xs = np.concatenate([obs[0:1], obs[:-1]], axis=0)[:, :, 0:2]  # [T,B,2]
    ph_full = np.asarray(pred_lstm_hidden, f32)

    common = dict(
        whh=whh_stat, wblk=wblk_stat,
        w0=np.ascontiguousarray(np.asarray(map_w0, f32)),
        w1=np.ascontiguousarray(np.asarray(map_w1, f32)),
        oww=np.ascontiguousarray(np.asarray(out_w, f32)),
        bpack=bpack, ident=np.eye(128, dtype=f32),
    )
    in_maps = []
    for c in range(NCORES):
        bs = slice(c * BC, (c + 1) * BC)
        ph_core = np.ascontiguousarray(
            ph_full[:, bs, :].reshape(ROWS, H))
        x_core = xs[:, bs, :]                       # [T, BC=256, 2]
        xt = np.empty((3, OBS_LEN, CHUNK), f32)     # x~ = (x0, x1, 1)
        xt[0] = x_core[:, :, 0]
        xt[1] = x_core[:, :, 1]
        xt[2] = 1.0
        # block-diagonal movings: (f,i) rows 0-5, (o,g) rows 6-11
        xmf_core = np.zeros((128, OBS_LEN, 2, CHUNK), f32)
        xmg_core = np.zeros((128, OBS_LEN, 2, CHUNK), f32)
        for half in range(2):
            xmf_core[3 * half:3 * half + 3, :, half, :] = xt
            xmg_core[6 + 3 * half:9 + 3 * half, :, half, :] = xt
        in_maps.append(dict(
            ph=ph_core,
            xmf=xmf_core.reshape(128, OBS_LEN, 2 * CHUNK),
            xmg=xmg_core.reshape(128, OBS_LEN, 2 * CHUNK), **common))
    return in_maps


def assemble_output(results):
    """Per-core [T, 2, ROWS] (k-major rows) -> full [T, K, B, 2]."""
    out = np.empty((OBS_LEN, K, B, NC_OUT), np.float32)
    for c, res in enumerate(results):
        o = res["out"].reshape(OBS_LEN, NC_OUT, K, BC)
        out[:, :, c * BC:(c + 1) * BC, :] = o.transpose(0, 2, 3, 1)
    return out


def kernel(**inputs):
    nc = build_nc(reps=1)
    in_maps = prep_inputs(**inputs)
    res = run_bass_kernel_spmd(nc, in_maps, core_ids=list(range(NCORES)))
    return assemble_output(res.results)


if __name__ == "__main__":
    import reference as R
    inputs = {k: np.asarray(v) for k, v in R.setup_inputs().items()}
    got = kernel(**inputs)
    import jax.numpy as jnp
    ref = np.asarray(
        R.reference(**{k: jnp.asarray(v) for k, v in inputs.items()}))
    err = np.abs(got - ref).max()
    rel = err / np.abs(ref).max()
    print(f"absmax={err:.4e} rel={rel:.4e}")
